# revision 25
# baseline (speedup 1.0000x reference)
"""Multi-head cross attention (B=4, LQ=1024, LK=2048, D=1024, H=16) on 8 trn2 cores.

Sharding: batch (4-way) x head-group (2-way, 8 heads each). Each core computes a
partial output Y_part = softmax(Q_hg K_hg^T/sqrt(dh) + mask) V_hg @ W_O[hg rows];
host sums the two head-group partials per batch.

Key tricks:
  - Host compacts the key/value sequence to the unmasked keys (the reference
    mask kills ~half of them), padded to a multiple of 128; padded rows are
    zeros + a -1e30 exp-bias. Program is compiled per padded-chunk-count.
  - Whole pipeline in fp16 (same PE rate as bf16, 8x less quantization noise;
    full-output error ~7e-4 vs the 2e-2 gate). fp8 was measured at 1.8-2.7e-2
    end-to-end - too close to the gate to ship.
  - Scores are computed transposed (S^T[k, q]) so the key mask is a
    per-partition bias folded into the exp on the scalar engine and P^T chunks
    feed the PV matmul directly. Head pairs share a feature chunk at
    partitions 0-63 / 64-127, so their score matmuls land in disjoint PE row
    groups (tile_position (0,0)/(64,0)) and run concurrently. A -4.0 shift in
    the exp bias keeps exp bounded (scores reach ~7.9 on this data).
  - V is augmented with a ones column per head; the PV matmul then emits the
    softmax denominators as PSUM row 64. The reciprocal is partition-broadcast
    with a single SBUF->SBUF stride-0 DMA. The output bias is folded into the
    W_O matmul as a ones-row accumulation, so the y path is copy+DMA only.
  - The j loop is software-pipelined (PV lags scores by one step) so the PE
    never waits on the ACT exp; projections and W_O tiles are interleaved as
    fillers. The q-half sweep is outermost so the W_O tiles for the first
    q half run as fillers during the second half. PSUM->SBUF drains of the
    attention accumulators run on the ACT engine (frees the PSUM pool without
    waiting on the vector queue).
  - Input DMAs are token-sliced and split across the two hardware queues
    (sync/gpsimd) so the first khat/qhat slices start after ~1MB instead of
    after the full input load.
"""

import math
import numpy as np

import concourse.bass as bass
import concourse.mybir as mybir
from concourse import bacc
from concourse.tile import TileContext
from concourse.bass_utils import run_bass_kernel_spmd

F16 = mybir.dt.float16
F32 = mybir.dt.float32
NP_F16 = np.float16

B, LQ, LK, D = 4, 1024, 2048, 1024
H, DH = 16, 64
N_CORES = 8
HPC = 8            # heads per core
DC = HPC * DH      # 512 local feature dim
DCH = DC // 128    # 4 dc chunks (also head-pair count)
DK = D // 128      # 8 contraction chunks
QT = LQ // 128     # 8 query tiles
E = DH + 1         # augmented V width per head
NEG = -1.0e30
SHIFT = -4.0       # folded into exp bias (softmax shift-invariant)

_CACHE = {}


def _build_program(KT):
    """Build + compile the SPMD program for KT 128-wide key chunks."""
    LKP = KT * 128
    nc = bacc.Bacc("TRN2", target_bir_lowering=False, debug=False, num_devices=N_CORES)

    qT_d = nc.dram_tensor("qT", [128, DK, LQ], F16, kind="ExternalInput")
    kvT_d = nc.dram_tensor("kvT", [128, DK, LKP], F16, kind="ExternalInput")
    wq_d = nc.dram_tensor("wq", [DCH, 128, DK, 128], F16, kind="ExternalInput")
    wk_d = nc.dram_tensor("wk", [DCH, 128, DK, 128], F16, kind="ExternalInput")
    wv_d = nc.dram_tensor("wv", [128, DK, DC], F16, kind="ExternalInput")
    wo_d = nc.dram_tensor("wo", [128, DCH, D], F16, kind="ExternalInput")
    bq_d = nc.dram_tensor("bq", [128, DCH], F32, kind="ExternalInput")
    mask_d = nc.dram_tensor("maskb", [128, KT], F32, kind="ExternalInput")
    y_d = nc.dram_tensor("y", [LQ, D], F16, kind="ExternalOutput")


    with TileContext(nc) as tc:
        with (
            tc.tile_pool(name="consts", bufs=1) as consts,
            tc.tile_pool(name="ps", bufs=2, space="PSUM") as psum_big,
            tc.tile_pool(name="pso", bufs=4, space="PSUM") as psum_o,
            tc.tile_pool(name="exps", bufs=8) as exps_pool,
            tc.tile_pool(name="small", bufs=4) as small,
            tc.tile_pool(name="yout", bufs=2) as yout,
        ):
            kvT_in = consts.tile([128, DK, LKP], F16, name="kvT_in")
            wk_sb = consts.tile([128, DCH, DK * 128], F16, name="wk_sb")
            qT_in = consts.tile([128, DK, LQ], F16, name="qT_in")
            wq_sb = consts.tile([128, DCH, DK * 128], F16, name="wq_sb")
            wv_sb = consts.tile([128, DK, DC], F16, name="wv_sb")
            wo_sb = consts.tile([128, DCH, D], F16, name="wo_sb")
            mask_sb = consts.tile([128, KT], F32, name="mask_sb")
            bq_sb = consts.tile([128, DCH], F32, name="bq_sb")

            # ---- input DMAs: three rings, partition-major fat descriptors ----
            def wcblock(eng, sb, dram, c):
                eng.dma_start(out=sb[:, c, :], in_=dram[c])
            # sync ring (fast HWDGE): all of kvT and qT
            nc.sync.dma_start(out=kvT_in[:, 0:6, :], in_=kvT_d[:, 0:6, :])
            nc.sync.dma_start(out=kvT_in[:, 6:8, :], in_=kvT_d[:, 6:8, :])
            nc.sync.dma_start(out=qT_in[:, 0:4, :], in_=qT_d[:, 0:4, :])
            nc.sync.dma_start(out=qT_in[:, 4:8, :], in_=qT_d[:, 4:8, :])
            # gpsimd ring (slow SWDGE): wk c0, wq c0 (small, early)
            wcblock(nc.gpsimd, wk_sb, wk_d, 0)
            wcblock(nc.gpsimd, wq_sb, wq_d, 0)
            # scalar ring: mask/bq then wv (needed ~25us), late weights
            nc.scalar.dma_start(out=mask_sb, in_=mask_d[:])
            nc.scalar.dma_start(out=bq_sb, in_=bq_d[:])
            nc.scalar.dma_start(out=wv_sb, in_=wv_d[:])
            for c in range(1, DCH):
                wcblock(nc.scalar, wk_sb, wk_d, c)
            for c in range(1, DCH):
                wcblock(nc.scalar, wq_sb, wq_d, c)
            nc.scalar.dma_start(out=wo_sb, in_=wo_d[:])

            # ---- persistent intermediates ----
            qhatT = consts.tile([128, DCH, LQ], F16, name="qhatT")      # [dc, lq]
            khatT = consts.tile([128, DCH, LKP], F16, name="khatT")     # [dc, lk]
            v_sb = consts.tile([128, KT, HPC * E], F16, name="v_sb")    # v | ones
            onormT = consts.tile([128, DCH, LQ], F16, name="onormT")    # [dc, lq]

            nc.vector.memset(
                v_sb.rearrange("p t (h e) -> p t h e", e=E)[:, :, :, DH:DH + 1], 1.0
            )

            def khat_slice(c, n0):
                w = min(512, LKP - n0)
                ps = psum_o.tile([128, w], F32, name=f"ps_k{c}_{n0}", tag="pso")
                for d in range(DK):
                    nc.tensor.matmul(
                        ps,
                        lhsT=wk_sb[:, c, d * 128:(d + 1) * 128],
                        rhs=kvT_in[:, d, n0:n0 + w],
                        start=(d == 0), stop=(d == DK - 1),
                    )
                nc.vector.tensor_copy(out=khatT[:, c, n0:n0 + w], in_=ps)

            def qhat_half(c, nn):
                ps = psum_o.tile([128, 512], F32, name=f"ps_q{c}_{nn}", tag="pso")
                for d in range(DK):
                    nc.tensor.matmul(
                        ps,
                        lhsT=wq_sb[:, c, d * 128:(d + 1) * 128],
                        rhs=qT_in[:, d, nn:nn + 512],
                        start=(d == 0), stop=(d == DK - 1),
                    )
                nc.vector.tensor_scalar_add(
                    out=qhatT[:, c, nn:nn + 512], in0=ps, scalar1=bq_sb[:, c:c + 1]
                )

            def khat_chunk(c):
                for n0 in range(0, LKP, 512):
                    khat_slice(c, n0)

            def qhat_chunk(c):
                for nn in range(0, LQ, 512):
                    qhat_half(c, nn)

            def v_chunk(t):
                ps = psum_o.tile([128, DC], F32, name=f"ps_v{t}", tag="pso")
                for d in range(DK):
                    nc.tensor.matmul(
                        ps,
                        lhsT=kvT_in[:, d, t * 128:(t + 1) * 128],
                        rhs=wv_sb[:, d, :],
                        start=(d == 0), stop=(d == DK - 1),
                    )
                nc.vector.tensor_copy(
                    out=v_sb[:, t, :].rearrange("p (h e) -> p h e", e=E)[:, :, 0:DH],
                    in_=ps.rearrange("p (h e) -> p h e", e=DH),
                )

            def attention_pair(hp, qh, fillers=()):
                """One head pair (2 heads at partitions 0-63/64-127), one q half.
                j loop is software-pipelined: PV for step j is emitted after the
                scores for step j+1, so exp latency never stalls the PE."""
                fillers = list(fillers)
                h0, h1 = 2 * hp, 2 * hp + 1
                q0 = qh * 512
                opsA = psum_o.tile([E, 512], F32, name=f"opsA{hp}_{qh}", tag="pso")
                opsB = psum_o.tile([E, 512], F32, name=f"opsB{hp}_{qh}", tag="pso")
                NJP = (KT + 1) // 2
                es_t = [None] * NJP

                def pv_quad(jp):
                    # one es-wait covers up to 4 matmuls (pipelined entries)
                    for h, ops, o0 in ((h0, opsA, 0), (h1, opsB, 512)):
                        for i in range(2 if 2 * jp + 1 < KT else 1):
                            j = 2 * jp + i
                            nc.tensor.matmul(
                                ops,
                                lhsT=v_sb[:, j, h * E:(h + 1) * E],
                                rhs=es_t[jp][:, i, o0:o0 + 512],
                                start=(j == 0), stop=(j == KT - 1),
                            )

                def scores_exp(j):
                    ps = psum_big.tile([128, 1024], F32, name=f"ps_s{hp}_{qh}_{j}", tag="ss")
                    # head pair in disjoint PE row groups -> concurrent
                    nc.tensor.matmul(
                        ps[:, 0:512],
                        lhsT=khatT[0:64, hp, j * 128:(j + 1) * 128],
                        rhs=qhatT[0:64, hp, q0:q0 + 512],
                        start=True, stop=True,
                    )
                    nc.tensor.matmul(
                        ps[:, 512:1024],
                        lhsT=khatT[64:128, hp, j * 128:(j + 1) * 128],
                        rhs=qhatT[64:128, hp, q0:q0 + 512],
                        start=True, stop=True,
                    )
                    if fillers:
                        fillers.pop(0)()
                    jp, i = j // 2, j % 2
                    if i == 0:
                        es_t[jp] = exps_pool.tile(
                            [128, 2, 1024], F16, name=f"es{hp}_{qh}_{jp}", tag="es")
                    nc.scalar.activation(
                        out=es_t[jp][:, i, :], in_=ps,
                        func=mybir.ActivationFunctionType.Exp,
                        bias=mask_sb[:, j:j + 1], scale=1.0,
                    )

                for j in range(KT):
                    scores_exp(j)
                    if j % 2 == 1 and j >= 3:
                        pv_quad(j // 2 - 1)
                if KT % 2 == 1:
                    pv_quad((KT - 1) // 2 - 1)
                pv_quad((KT - 1) // 2)

                for h, po, ops in ((h0, 0, opsA), (h1, 64, opsB)):
                    # drain PSUM on ACT (frees the pso slot without waiting on
                    # the busy vector queue)
                    ou = small.tile([E, 512], F32, name=f"ou{h}_{qh}", tag="ou")
                    nc.scalar.copy(out=ou, in_=ops)
                    den = small.tile([1, 512], F32, name=f"den{h}_{qh}", tag="den")
                    nc.vector.tensor_copy(out=den, in_=ou[DH:DH + 1, :])
                    rec = small.tile([1, 512], F32, name=f"rec{h}_{qh}", tag="rec")
                    nc.vector.reciprocal_approx_fast(out=rec, in_=den)
                    rbc = small.tile([64, 512], F32, name=f"rbc{h}_{qh}", tag="rbc")
                    nc.gpsimd.partition_broadcast(rbc, rec)
                    nc.vector.tensor_mul(
                        out=onormT[po:po + 64, hp, q0:q0 + 512],
                        in0=ou[0:DH, :], in1=rbc,
                    )
                while fillers:
                    fillers.pop(0)()

            def wo_tile(m, drain):
                ys = yout.tile([128, 1024], F16, name=f"ys{m}", tag="ys")
                for n in range(2):
                    ps = psum_o.tile([128, 512], F32, name=f"ps_y{m}_{n}", tag="pso")
                    for c in range(DCH):
                        nc.tensor.matmul(
                            ps,
                            lhsT=onormT[:, c, m * 128:(m + 1) * 128],
                            rhs=wo_sb[:, c, n * 512:(n + 1) * 512],
                            start=(c == 0), stop=(c == DCH - 1),
                        )
                    drain(out=ys[:, n * 512:(n + 1) * 512], in_=ps)
                eng = nc.sync if m % 2 == 0 else nc.scalar
                eng.dma_start(out=y_d[m * 128:(m + 1) * 128, :], in_=ys)

            # ---- emission order ----
            khat_slice(0, 0)
            qhat_half(0, 0)

            def proj_fillers(c):
                f = [lambda n0=n0: khat_slice(c, n0) for n0 in range(0, LKP, 512)]
                f += [lambda nn=nn: qhat_half(c, nn) for nn in range(0, LQ, 512)]
                return f

            # qh=0 sweep: pair hp's fillers prefetch chunk hp+1's projections
            # (pair 0 also computes the V chunks, needed by its own PV steps).
            lead = [lambda n0=n0: khat_slice(0, n0) for n0 in range(512, LKP, 512)]
            lead += proj_fillers(1)
            lead += [lambda: qhat_half(0, 512)]

            def slot0(t, extras):
                def f():
                    v_chunk(t)
                    for e in extras:
                        e()
                return f

            nlead = len(lead)
            p0f = []
            for t in range(KT):
                take = lead[:2] if t < nlead / 2 + 1 else lead[:1]
                lead = lead[len(take):]
                p0f.append(slot0(t, take))
            p0f += lead
            attention_pair(0, 0, fillers=p0f)
            for hp in range(1, DCH):
                f = proj_fillers(hp + 1) if hp + 1 < DCH else []
                attention_pair(hp, 0, fillers=f)
            # qh=1 sweep: fill with W_O tiles for the first q half (PSUM drain
            # on the vector engine; ACT is busy with exp there).
            wof = [
                lambda m=m: wo_tile(m, nc.vector.tensor_copy)
                for m in range(QT // 2)
            ]
            attention_pair(0, 1, fillers=wof[0:1])
            attention_pair(1, 1, fillers=wof[1:2])
            attention_pair(2, 1, fillers=wof[2:3])
            attention_pair(3, 1, fillers=wof[3:4])
            # tail: W_O tiles for the second q half (drains split vector/ACT)
            for m in range(QT // 2, QT):
                drain = nc.vector.tensor_copy if m % 2 == 0 else (
                    lambda out, in_: nc.scalar.copy(out=out, in_=in_))
                wo_tile(m, drain)

    nc.compile()
    return nc


def _get_program(KT):
    key = ("nc", KT)
    if key not in _CACHE:
        _CACHE[key] = _build_program(KT)
    return _CACHE[key]


def kernel(q_input, kv_input, key_padding_mask, W_Q, b_Q, W_K, b_K, W_V, b_V, W_O, b_O):
    q_input = np.asarray(q_input, dtype=np.float32)
    kv_input = np.asarray(kv_input, dtype=np.float32)
    key_padding_mask = np.asarray(key_padding_mask).astype(bool)
    W_Q = np.asarray(W_Q, dtype=np.float32)
    b_Q = np.asarray(b_Q, dtype=np.float32)
    W_K = np.asarray(W_K, dtype=np.float32)
    W_V = np.asarray(W_V, dtype=np.float32)
    b_V = np.asarray(b_V, dtype=np.float32)
    W_O = np.asarray(W_O, dtype=np.float32)
    b_O = np.asarray(b_O, dtype=np.float32)

    # compact keys/values to the unmasked rows, pad to a 128 multiple
    keeps = [~key_padding_mask[b] for b in range(B)]
    effs = [int(k.sum()) for k in keeps]
    KT = max(2, math.ceil(max(effs) / 128))
    LKP = KT * 128
    nc = _get_program(KT)

    # per head-group constants
    hg_const = []
    for hg in range(2):
        sl = slice(hg * DC, (hg + 1) * DC)
        wq = np.ascontiguousarray(
            (W_Q[:, sl] * 0.125).astype(NP_F16).reshape(DK, 128, DCH, 128).transpose(2, 1, 0, 3))
        wk = np.ascontiguousarray(
            W_K[:, sl].astype(NP_F16).reshape(DK, 128, DCH, 128).transpose(2, 1, 0, 3))
        wv = np.ascontiguousarray(W_V[:, sl].astype(NP_F16).reshape(DK, 128, DC).transpose(1, 0, 2))
        wo = np.ascontiguousarray(W_O[sl, :].astype(NP_F16).reshape(DCH, 128, D).transpose(1, 0, 2))
        bq = np.ascontiguousarray((b_Q[sl] * 0.125).astype(np.float32).reshape(DCH, 128).T)
        hg_const.append((wq, wk, wv, wo, bq))

    per_batch = []
    for b in range(B):
        kvcT = kv_input[b][keeps[b]].T       # [D, eff] f32
        kvT = np.zeros((128, DK, LKP), NP_F16)
        kvT[:, :, :effs[b]] = kvcT.astype(NP_F16).reshape(DK, 128, effs[b]).transpose(1, 0, 2)
        maskb = np.full(KT * 128, np.float32(NEG), np.float32)
        maskb[:effs[b]] = np.float32(SHIFT)
        mask2 = np.ascontiguousarray(maskb.reshape(KT, 128).T)   # [128, KT]
        qTp = np.ascontiguousarray(
            q_input[b].T.astype(NP_F16).reshape(DK, 128, LQ).transpose(1, 0, 2))
        per_batch.append((qTp, kvT, mask2))

    in_maps = []
    for core in range(N_CORES):
        b, hg = core // 2, core % 2
        wq, wk, wv, wo, bq = hg_const[hg]
        qT, kvT, mask2 = per_batch[b]
        in_maps.append({
            "qT": qT, "kvT": kvT,
            "wq": wq, "wk": wk, "wv": wv, "wo": wo,
            "bq": bq, "maskb": mask2,
        })

    _CACHE["in_maps"] = in_maps
    _CACHE["last_KT"] = KT
    res = run_bass_kernel_spmd(nc, in_maps, core_ids=list(range(N_CORES)))
    bias_full = (b_V.astype(np.float64) @ W_O.astype(np.float64) + b_O).astype(np.float32)
    out = np.stack(
        [res.results[2 * b]["y"].astype(np.float32) + res.results[2 * b + 1]["y"].astype(np.float32)
         for b in range(B)]
    ) + bias_full
    return out.astype(np.float32)


# revision 26
# speedup vs baseline: 1.0191x; 1.0191x over previous
"""Multi-head cross attention (B=4, LQ=1024, LK=2048, D=1024, H=16) on 8 trn2 cores.

Sharding: batch (4-way) x head-group (2-way, 8 heads each). Each core computes a
partial output Y_part = softmax(Q_hg K_hg^T/sqrt(dh) + mask) V_hg @ W_O[hg rows];
host sums the two head-group partials per batch.

Key tricks:
  - Host compacts the key/value sequence to the unmasked keys (the reference
    mask kills ~half of them), padded to a multiple of 128; padded rows are
    zeros + a -1e30 exp-bias. Program is compiled per padded-chunk-count.
  - Whole pipeline in fp16 (same PE rate as bf16, 8x less quantization noise;
    full-output error ~7e-4 vs the 2e-2 gate). fp8 was measured at 1.8-2.7e-2
    end-to-end - too close to the gate to ship.
  - Scores are computed transposed (S^T[k, q]) so the key mask is a
    per-partition bias folded into the exp on the scalar engine and P^T chunks
    feed the PV matmul directly. Head pairs share a feature chunk at
    partitions 0-63 / 64-127, so their score matmuls land in disjoint PE row
    groups (tile_position (0,0)/(64,0)) and run concurrently. A -4.0 shift in
    the exp bias keeps exp bounded (scores reach ~7.9 on this data).
  - V is augmented with a ones column per head; the PV matmul then emits the
    softmax denominators as PSUM row 64. The reciprocal is partition-broadcast
    with a single SBUF->SBUF stride-0 DMA. The output bias is folded into the
    W_O matmul as a ones-row accumulation, so the y path is copy+DMA only.
  - The j loop is software-pipelined (PV lags scores by one step) so the PE
    never waits on the ACT exp; projections and W_O tiles are interleaved as
    fillers. The q-half sweep is outermost so the W_O tiles for the first
    q half run as fillers during the second half. PSUM->SBUF drains of the
    attention accumulators run on the ACT engine (frees the PSUM pool without
    waiting on the vector queue).
  - Input DMAs are token-sliced and split across the two hardware queues
    (sync/gpsimd) so the first khat/qhat slices start after ~1MB instead of
    after the full input load.
"""

import math
import numpy as np

import concourse.bass as bass
import concourse.mybir as mybir
from concourse import bacc
from concourse.tile import TileContext
from concourse.bass_utils import run_bass_kernel_spmd

F16 = mybir.dt.float16
F32 = mybir.dt.float32
NP_F16 = np.float16

B, LQ, LK, D = 4, 1024, 2048, 1024
H, DH = 16, 64
N_CORES = 8
HPC = 8            # heads per core
DC = HPC * DH      # 512 local feature dim
DCH = DC // 128    # 4 dc chunks (also head-pair count)
DK = D // 128      # 8 contraction chunks
QT = LQ // 128     # 8 query tiles
E = DH + 1         # augmented V width per head
NEG = -1.0e30
SHIFT = -4.0       # folded into exp bias (softmax shift-invariant)

_CACHE = {}


def _build_program(KT):
    """Build + compile the SPMD program for KT 128-wide key chunks."""
    LKP = KT * 128
    nc = bacc.Bacc("TRN2", target_bir_lowering=False, debug=False, num_devices=N_CORES)

    qT_d = nc.dram_tensor("qT", [128, DK, LQ], F16, kind="ExternalInput")
    kvT_d = nc.dram_tensor("kvT", [128, DK, LKP], F16, kind="ExternalInput")
    wq_d = nc.dram_tensor("wq", [DCH, 128, DK, 128], F16, kind="ExternalInput")
    wk_d = nc.dram_tensor("wk", [DCH, 128, DK, 128], F16, kind="ExternalInput")
    wv_d = nc.dram_tensor("wv", [128, DK, DC], F16, kind="ExternalInput")
    wo_d = nc.dram_tensor("wo", [128, DCH, D], F16, kind="ExternalInput")
    bq_d = nc.dram_tensor("bq", [128, DCH], F32, kind="ExternalInput")
    mask_d = nc.dram_tensor("maskb", [128, KT], F32, kind="ExternalInput")
    y_d = nc.dram_tensor("y", [LQ, D], F16, kind="ExternalOutput")


    with TileContext(nc) as tc:
        with (
            tc.tile_pool(name="consts", bufs=1) as consts,
            tc.tile_pool(name="ps", bufs=2, space="PSUM") as psum_big,
            tc.tile_pool(name="pso", bufs=4, space="PSUM") as psum_o,
            tc.tile_pool(name="exps", bufs=8) as exps_pool,
            tc.tile_pool(name="small", bufs=4) as small,
            tc.tile_pool(name="yout", bufs=2) as yout,
        ):
            kvT_in = consts.tile([128, DK, LKP], F16, name="kvT_in")
            wk_sb = consts.tile([128, DCH, DK * 128], F16, name="wk_sb")
            qT_in = consts.tile([128, DK, LQ], F16, name="qT_in")
            wq_sb = consts.tile([128, DCH, DK * 128], F16, name="wq_sb")
            wv_sb = consts.tile([128, DK, DC], F16, name="wv_sb")
            wo_sb = consts.tile([128, DCH, D], F16, name="wo_sb")
            mask_sb = consts.tile([128, KT], F32, name="mask_sb")
            bq_sb = consts.tile([128, DCH], F32, name="bq_sb")

            # ---- input DMAs: three rings, partition-major fat descriptors ----
            def wcblock(eng, sb, dram, c):
                eng.dma_start(out=sb[:, c, :], in_=dram[c])
            # sync ring (fast HWDGE): kvT c0-5, qT c0-3, wv, qT c4-7
            nc.sync.dma_start(out=kvT_in[:, 0:6, :], in_=kvT_d[:, 0:6, :])
            nc.sync.dma_start(out=qT_in[:, 0:4, :], in_=qT_d[:, 0:4, :])
            nc.sync.dma_start(out=wv_sb, in_=wv_d[:])
            nc.sync.dma_start(out=qT_in[:, 4:8, :], in_=qT_d[:, 4:8, :])
            # gpsimd ring (slow SWDGE): wk c0, kvT c6-7
            wcblock(nc.gpsimd, wk_sb, wk_d, 0)
            nc.gpsimd.dma_start(out=kvT_in[:, 6:8, :], in_=kvT_d[:, 6:8, :])
            # scalar ring: mask/bq/wq-c0 (small, early), then late weights
            nc.scalar.dma_start(out=mask_sb, in_=mask_d[:])
            nc.scalar.dma_start(out=bq_sb, in_=bq_d[:])
            wcblock(nc.scalar, wq_sb, wq_d, 0)
            for c in range(1, DCH):
                wcblock(nc.scalar, wk_sb, wk_d, c)
            for c in range(1, DCH):
                wcblock(nc.scalar, wq_sb, wq_d, c)
            nc.scalar.dma_start(out=wo_sb, in_=wo_d[:])

            # ---- persistent intermediates ----
            qhatT = consts.tile([128, DCH, LQ], F16, name="qhatT")      # [dc, lq]
            khatT = consts.tile([128, DCH, LKP], F16, name="khatT")     # [dc, lk]
            v_sb = consts.tile([128, KT, HPC * E], F16, name="v_sb")    # v | ones
            onormT = consts.tile([128, DCH, LQ], F16, name="onormT")    # [dc, lq]

            nc.vector.memset(
                v_sb.rearrange("p t (h e) -> p t h e", e=E)[:, :, :, DH:DH + 1], 1.0
            )

            def khat_slice(c, n0):
                w = min(512, LKP - n0)
                ps = psum_o.tile([128, w], F32, name=f"ps_k{c}_{n0}", tag="pso")
                for d in range(DK):
                    nc.tensor.matmul(
                        ps,
                        lhsT=wk_sb[:, c, d * 128:(d + 1) * 128],
                        rhs=kvT_in[:, d, n0:n0 + w],
                        start=(d == 0), stop=(d == DK - 1),
                    )
                nc.vector.tensor_copy(out=khatT[:, c, n0:n0 + w], in_=ps)

            def qhat_half(c, nn):
                ps = psum_o.tile([128, 512], F32, name=f"ps_q{c}_{nn}", tag="pso")
                for d in range(DK):
                    nc.tensor.matmul(
                        ps,
                        lhsT=wq_sb[:, c, d * 128:(d + 1) * 128],
                        rhs=qT_in[:, d, nn:nn + 512],
                        start=(d == 0), stop=(d == DK - 1),
                    )
                nc.vector.tensor_scalar_add(
                    out=qhatT[:, c, nn:nn + 512], in0=ps, scalar1=bq_sb[:, c:c + 1]
                )

            def khat_chunk(c):
                for n0 in range(0, LKP, 512):
                    khat_slice(c, n0)

            def qhat_chunk(c):
                for nn in range(0, LQ, 512):
                    qhat_half(c, nn)

            def v_chunk(t):
                ps = psum_o.tile([128, DC], F32, name=f"ps_v{t}", tag="pso")
                for d in range(DK):
                    nc.tensor.matmul(
                        ps,
                        lhsT=kvT_in[:, d, t * 128:(t + 1) * 128],
                        rhs=wv_sb[:, d, :],
                        start=(d == 0), stop=(d == DK - 1),
                    )
                nc.vector.tensor_copy(
                    out=v_sb[:, t, :].rearrange("p (h e) -> p h e", e=E)[:, :, 0:DH],
                    in_=ps.rearrange("p (h e) -> p h e", e=DH),
                )

            def attention_pair(hp, qh, fillers=()):
                """One head pair (2 heads at partitions 0-63/64-127), one q half.
                j loop is software-pipelined: PV for step j is emitted after the
                scores for step j+1, so exp latency never stalls the PE."""
                fillers = list(fillers)
                h0, h1 = 2 * hp, 2 * hp + 1
                q0 = qh * 512
                opsA = psum_o.tile([E, 512], F32, name=f"opsA{hp}_{qh}", tag="pso")
                opsB = psum_o.tile([E, 512], F32, name=f"opsB{hp}_{qh}", tag="pso")
                NJP = (KT + 1) // 2
                es_t = [None] * NJP

                def pv_quad(jp):
                    # one es-wait covers up to 4 matmuls (pipelined entries)
                    for h, ops, o0 in ((h0, opsA, 0), (h1, opsB, 512)):
                        for i in range(2 if 2 * jp + 1 < KT else 1):
                            j = 2 * jp + i
                            nc.tensor.matmul(
                                ops,
                                lhsT=v_sb[:, j, h * E:(h + 1) * E],
                                rhs=es_t[jp][:, i, o0:o0 + 512],
                                start=(j == 0), stop=(j == KT - 1),
                            )

                def scores_exp(j):
                    ps = psum_big.tile([128, 1024], F32, name=f"ps_s{hp}_{qh}_{j}", tag="ss")
                    # head pair in disjoint PE row groups -> concurrent
                    nc.tensor.matmul(
                        ps[:, 0:512],
                        lhsT=khatT[0:64, hp, j * 128:(j + 1) * 128],
                        rhs=qhatT[0:64, hp, q0:q0 + 512],
                        start=True, stop=True,
                    )
                    nc.tensor.matmul(
                        ps[:, 512:1024],
                        lhsT=khatT[64:128, hp, j * 128:(j + 1) * 128],
                        rhs=qhatT[64:128, hp, q0:q0 + 512],
                        start=True, stop=True,
                    )
                    if fillers:
                        fillers.pop(0)()
                    jp, i = j // 2, j % 2
                    if i == 0:
                        es_t[jp] = exps_pool.tile(
                            [128, 2, 1024], F16, name=f"es{hp}_{qh}_{jp}", tag="es")
                    nc.scalar.activation(
                        out=es_t[jp][:, i, :], in_=ps,
                        func=mybir.ActivationFunctionType.Exp,
                        bias=mask_sb[:, j:j + 1], scale=1.0,
                    )

                for j in range(KT):
                    scores_exp(j)
                    if j % 2 == 1 and j >= 3:
                        pv_quad(j // 2 - 1)
                if KT % 2 == 1:
                    pv_quad((KT - 1) // 2 - 1)
                pv_quad((KT - 1) // 2)

                for h, po, ops in ((h0, 0, opsA), (h1, 64, opsB)):
                    # drain PSUM on ACT (frees the pso slot without waiting on
                    # the busy vector queue)
                    ou = small.tile([E, 512], F32, name=f"ou{h}_{qh}", tag="ou")
                    nc.scalar.copy(out=ou, in_=ops)
                    den = small.tile([1, 512], F32, name=f"den{h}_{qh}", tag="den")
                    nc.vector.tensor_copy(out=den, in_=ou[DH:DH + 1, :])
                    rec = small.tile([1, 512], F32, name=f"rec{h}_{qh}", tag="rec")
                    nc.vector.reciprocal_approx_fast(out=rec, in_=den)
                    rbc = small.tile([64, 512], F32, name=f"rbc{h}_{qh}", tag="rbc")
                    nc.gpsimd.partition_broadcast(rbc, rec)
                    nc.vector.tensor_mul(
                        out=onormT[po:po + 64, hp, q0:q0 + 512],
                        in0=ou[0:DH, :], in1=rbc,
                    )
                while fillers:
                    fillers.pop(0)()

            def wo_tile(m, drain):
                ys = yout.tile([128, 1024], F16, name=f"ys{m}", tag="ys")
                for n in range(2):
                    ps = psum_o.tile([128, 512], F32, name=f"ps_y{m}_{n}", tag="pso")
                    for c in range(DCH):
                        nc.tensor.matmul(
                            ps,
                            lhsT=onormT[:, c, m * 128:(m + 1) * 128],
                            rhs=wo_sb[:, c, n * 512:(n + 1) * 512],
                            start=(c == 0), stop=(c == DCH - 1),
                        )
                    drain(out=ys[:, n * 512:(n + 1) * 512], in_=ps)
                eng = nc.sync if m % 2 == 0 else nc.scalar
                eng.dma_start(out=y_d[m * 128:(m + 1) * 128, :], in_=ys)

            # ---- emission order ----
            khat_slice(0, 0)
            qhat_half(0, 0)

            def proj_fillers(c):
                f = [lambda n0=n0: khat_slice(c, n0) for n0 in range(0, LKP, 512)]
                f += [lambda nn=nn: qhat_half(c, nn) for nn in range(0, LQ, 512)]
                return f

            # qh=0 sweep: pair hp's fillers prefetch chunk hp+1's projections
            # (pair 0 also computes the V chunks, needed by its own PV steps).
            lead = [lambda n0=n0: khat_slice(0, n0) for n0 in range(512, LKP, 512)]
            lead += proj_fillers(1)
            lead += [lambda: qhat_half(0, 512)]

            def slot0(t, extras):
                def f():
                    v_chunk(t)
                    for e in extras:
                        e()
                return f

            nlead = len(lead)
            p0f = []
            for t in range(KT):
                take = lead[:2] if t < nlead / 2 + 1 else lead[:1]
                lead = lead[len(take):]
                p0f.append(slot0(t, take))
            p0f += lead
            attention_pair(0, 0, fillers=p0f)
            for hp in range(1, DCH):
                f = proj_fillers(hp + 1) if hp + 1 < DCH else []
                attention_pair(hp, 0, fillers=f)
            # qh=1 sweep: fill with W_O tiles for the first q half (PSUM drain
            # on the vector engine; ACT is busy with exp there).
            wof = [
                lambda m=m: wo_tile(m, nc.vector.tensor_copy)
                for m in range(QT // 2)
            ]
            attention_pair(0, 1, fillers=wof[0:1])
            attention_pair(1, 1, fillers=wof[1:2])
            attention_pair(2, 1, fillers=wof[2:3])
            attention_pair(3, 1, fillers=wof[3:4])
            # tail: W_O tiles for the second q half (drains split vector/ACT)
            for m in range(QT // 2, QT):
                drain = nc.vector.tensor_copy if m % 2 == 0 else (
                    lambda out, in_: nc.scalar.copy(out=out, in_=in_))
                wo_tile(m, drain)

    nc.compile()
    return nc


def _get_program(KT):
    key = ("nc", KT)
    if key not in _CACHE:
        _CACHE[key] = _build_program(KT)
    return _CACHE[key]


def kernel(q_input, kv_input, key_padding_mask, W_Q, b_Q, W_K, b_K, W_V, b_V, W_O, b_O):
    q_input = np.asarray(q_input, dtype=np.float32)
    kv_input = np.asarray(kv_input, dtype=np.float32)
    key_padding_mask = np.asarray(key_padding_mask).astype(bool)
    W_Q = np.asarray(W_Q, dtype=np.float32)
    b_Q = np.asarray(b_Q, dtype=np.float32)
    W_K = np.asarray(W_K, dtype=np.float32)
    W_V = np.asarray(W_V, dtype=np.float32)
    b_V = np.asarray(b_V, dtype=np.float32)
    W_O = np.asarray(W_O, dtype=np.float32)
    b_O = np.asarray(b_O, dtype=np.float32)

    # compact keys/values to the unmasked rows, pad to a 128 multiple
    keeps = [~key_padding_mask[b] for b in range(B)]
    effs = [int(k.sum()) for k in keeps]
    KT = max(2, math.ceil(max(effs) / 128))
    LKP = KT * 128
    nc = _get_program(KT)

    # per head-group constants
    hg_const = []
    for hg in range(2):
        sl = slice(hg * DC, (hg + 1) * DC)
        wq = np.ascontiguousarray(
            (W_Q[:, sl] * 0.125).astype(NP_F16).reshape(DK, 128, DCH, 128).transpose(2, 1, 0, 3))
        wk = np.ascontiguousarray(
            W_K[:, sl].astype(NP_F16).reshape(DK, 128, DCH, 128).transpose(2, 1, 0, 3))
        wv = np.ascontiguousarray(W_V[:, sl].astype(NP_F16).reshape(DK, 128, DC).transpose(1, 0, 2))
        wo = np.ascontiguousarray(W_O[sl, :].astype(NP_F16).reshape(DCH, 128, D).transpose(1, 0, 2))
        bq = np.ascontiguousarray((b_Q[sl] * 0.125).astype(np.float32).reshape(DCH, 128).T)
        hg_const.append((wq, wk, wv, wo, bq))

    per_batch = []
    for b in range(B):
        kvcT = kv_input[b][keeps[b]].T       # [D, eff] f32
        kvT = np.zeros((128, DK, LKP), NP_F16)
        kvT[:, :, :effs[b]] = kvcT.astype(NP_F16).reshape(DK, 128, effs[b]).transpose(1, 0, 2)
        maskb = np.full(KT * 128, np.float32(NEG), np.float32)
        maskb[:effs[b]] = np.float32(SHIFT)
        mask2 = np.ascontiguousarray(maskb.reshape(KT, 128).T)   # [128, KT]
        qTp = np.ascontiguousarray(
            q_input[b].T.astype(NP_F16).reshape(DK, 128, LQ).transpose(1, 0, 2))
        per_batch.append((qTp, kvT, mask2))

    in_maps = []
    for core in range(N_CORES):
        b, hg = core // 2, core % 2
        wq, wk, wv, wo, bq = hg_const[hg]
        qT, kvT, mask2 = per_batch[b]
        in_maps.append({
            "qT": qT, "kvT": kvT,
            "wq": wq, "wk": wk, "wv": wv, "wo": wo,
            "bq": bq, "maskb": mask2,
        })

    _CACHE["in_maps"] = in_maps
    _CACHE["last_KT"] = KT
    res = run_bass_kernel_spmd(nc, in_maps, core_ids=list(range(N_CORES)))
    bias_full = (b_V.astype(np.float64) @ W_O.astype(np.float64) + b_O).astype(np.float32)
    out = np.stack(
        [res.results[2 * b]["y"].astype(np.float32) + res.results[2 * b + 1]["y"].astype(np.float32)
         for b in range(B)]
    ) + bias_full
    return out.astype(np.float32)


# revision 27
# speedup vs baseline: 1.0208x; 1.0017x over previous
"""Multi-head cross attention (B=4, LQ=1024, LK=2048, D=1024, H=16) on 8 trn2 cores.

Sharding: batch (4-way) x head-group (2-way, 8 heads each). Each core computes a
partial output Y_part = softmax(Q_hg K_hg^T/sqrt(dh) + mask) V_hg @ W_O[hg rows];
host sums the two head-group partials per batch.

Key tricks:
  - Host compacts the key/value sequence to the unmasked keys (the reference
    mask kills ~half of them), padded to a multiple of 128; padded rows are
    zeros + a -1e30 exp-bias. Program is compiled per padded-chunk-count.
  - Whole pipeline in fp16 (same PE rate as bf16, 8x less quantization noise;
    full-output error ~7e-4 vs the 2e-2 gate). fp8 was measured at 1.8-2.7e-2
    end-to-end - too close to the gate to ship.
  - Scores are computed transposed (S^T[k, q]) so the key mask is a
    per-partition bias folded into the exp on the scalar engine and P^T chunks
    feed the PV matmul directly. Head pairs share a feature chunk at
    partitions 0-63 / 64-127, so their score matmuls land in disjoint PE row
    groups (tile_position (0,0)/(64,0)) and run concurrently. A -4.0 shift in
    the exp bias keeps exp bounded (scores reach ~7.9 on this data).
  - V is augmented with a ones column per head; the PV matmul then emits the
    softmax denominators as PSUM row 64. The reciprocal is broadcast across
    partitions with gpsimd.partition_broadcast. The output bias is added on
    the host (partials are summed there anyway); y returns as fp16.
  - The j loop is software-pipelined: PV matmuls are emitted in quads two
    steps behind the scores, so one es semaphore wait covers four matmuls
    and the PE never waits on the ACT exp. Projections and W_O tiles are
    interleaved as fillers; the q-half sweep is outermost so the W_O tiles
    for the first q half run as fillers during the second half. PSUM drains
    of the attention accumulators run on the ACT engine.
  - Inputs are host-packed partition-major (one contiguous run per partition
    per DMA) and scheduled critical-path-first across the three DMA queues;
    only the SP queue sustains ~190GB/s (Act ~55, gpsimd SWDGE ~50), so the
    kvT/qT bulk rides SP and small/late weights ride the others.
"""

import math
import numpy as np

import concourse.bass as bass
import concourse.mybir as mybir
from concourse import bacc
from concourse.tile import TileContext
from concourse.bass_utils import run_bass_kernel_spmd

F16 = mybir.dt.float16
F32 = mybir.dt.float32
NP_F16 = np.float16

B, LQ, LK, D = 4, 1024, 2048, 1024
H, DH = 16, 64
N_CORES = 8
HPC = 8            # heads per core
DC = HPC * DH      # 512 local feature dim
DCH = DC // 128    # 4 dc chunks (also head-pair count)
DK = D // 128      # 8 contraction chunks
QT = LQ // 128     # 8 query tiles
E = DH + 1         # augmented V width per head
NEG = -1.0e30
SHIFT = -4.0       # folded into exp bias (softmax shift-invariant)

_CACHE = {}


def _build_program(KT):
    """Build + compile the SPMD program for KT 128-wide key chunks."""
    LKP = KT * 128
    nc = bacc.Bacc("TRN2", target_bir_lowering=False, debug=False, num_devices=N_CORES)

    qT_d = nc.dram_tensor("qT", [128, DK, LQ], F16, kind="ExternalInput")
    kvT_d = nc.dram_tensor("kvT", [128, DK, LKP], F16, kind="ExternalInput")
    wq_d = nc.dram_tensor("wq", [DCH, 128, DK, 128], F16, kind="ExternalInput")
    wk_d = nc.dram_tensor("wk", [DCH, 128, DK, 128], F16, kind="ExternalInput")
    wv_d = nc.dram_tensor("wv", [128, DK, DC], F16, kind="ExternalInput")
    wo_d = nc.dram_tensor("wo", [128, DCH, D], F16, kind="ExternalInput")
    bq_d = nc.dram_tensor("bq", [128, DCH], F32, kind="ExternalInput")
    mask_d = nc.dram_tensor("maskb", [128, KT], F32, kind="ExternalInput")
    y_d = nc.dram_tensor("y", [LQ, D], F16, kind="ExternalOutput")


    with TileContext(nc) as tc:
        with (
            tc.tile_pool(name="consts", bufs=1) as consts,
            tc.tile_pool(name="ps", bufs=2, space="PSUM") as psum_big,
            tc.tile_pool(name="pso", bufs=4, space="PSUM") as psum_o,
            tc.tile_pool(name="exps", bufs=8) as exps_pool,
            tc.tile_pool(name="small", bufs=4) as small,
            tc.tile_pool(name="yout", bufs=2) as yout,
        ):
            kvT_in = consts.tile([128, DK, LKP], F16, name="kvT_in")
            wk_sb = consts.tile([128, DCH, DK * 128], F16, name="wk_sb")
            qT_in = consts.tile([128, DK, LQ], F16, name="qT_in")
            wq_sb = consts.tile([128, DCH, DK * 128], F16, name="wq_sb")
            wv_sb = consts.tile([128, DK, DC], F16, name="wv_sb")
            wo_sb = consts.tile([128, DCH, D], F16, name="wo_sb")
            mask_sb = consts.tile([128, KT], F32, name="mask_sb")
            bq_sb = consts.tile([128, DCH], F32, name="bq_sb")

            # ---- input DMAs: three rings, partition-major fat descriptors ----
            def wcblock(eng, sb, dram, c):
                eng.dma_start(out=sb[:, c, :], in_=dram[c])
            # sync ring (fast HWDGE): kvT c0-5, qT c0-3, wv, qT c4-7
            nc.sync.dma_start(out=kvT_in[:, 0:6, :], in_=kvT_d[:, 0:6, :])
            nc.sync.dma_start(out=qT_in[:, 0:4, :], in_=qT_d[:, 0:4, :])
            nc.sync.dma_start(out=wv_sb, in_=wv_d[:])
            nc.sync.dma_start(out=qT_in[:, 4:8, :], in_=qT_d[:, 4:8, :])
            # gpsimd ring (slow SWDGE): wk c0, kvT c6-7
            wcblock(nc.gpsimd, wk_sb, wk_d, 0)
            nc.gpsimd.dma_start(out=kvT_in[:, 6:8, :], in_=kvT_d[:, 6:8, :])
            # scalar ring: mask/bq/wq-c0 (small, early), then late weights
            nc.scalar.dma_start(out=mask_sb, in_=mask_d[:])
            nc.scalar.dma_start(out=bq_sb, in_=bq_d[:])
            wcblock(nc.scalar, wq_sb, wq_d, 0)
            for c in range(1, DCH):
                wcblock(nc.scalar, wk_sb, wk_d, c)
            for c in range(1, DCH):
                wcblock(nc.scalar, wq_sb, wq_d, c)
            nc.scalar.dma_start(out=wo_sb, in_=wo_d[:])

            # ---- persistent intermediates ----
            qhatT = consts.tile([128, DCH, LQ], F16, name="qhatT")      # [dc, lq]
            khatT = consts.tile([128, DCH, LKP], F16, name="khatT")     # [dc, lk]
            v_sb = consts.tile([128, KT, HPC * E], F16, name="v_sb")    # v | ones
            onormT = consts.tile([128, DCH, LQ], F16, name="onormT")    # [dc, lq]

            nc.vector.memset(
                v_sb.rearrange("p t (h e) -> p t h e", e=E)[:, :, :, DH:DH + 1], 1.0
            )

            def khat_slice(c, n0):
                w = min(512, LKP - n0)
                ps = psum_o.tile([128, w], F32, name=f"ps_k{c}_{n0}", tag="pso")
                for d in range(DK):
                    nc.tensor.matmul(
                        ps,
                        lhsT=wk_sb[:, c, d * 128:(d + 1) * 128],
                        rhs=kvT_in[:, d, n0:n0 + w],
                        start=(d == 0), stop=(d == DK - 1),
                    )
                nc.vector.tensor_copy(out=khatT[:, c, n0:n0 + w], in_=ps)

            def qhat_half(c, nn):
                ps = psum_o.tile([128, 512], F32, name=f"ps_q{c}_{nn}", tag="pso")
                for d in range(DK):
                    nc.tensor.matmul(
                        ps,
                        lhsT=wq_sb[:, c, d * 128:(d + 1) * 128],
                        rhs=qT_in[:, d, nn:nn + 512],
                        start=(d == 0), stop=(d == DK - 1),
                    )
                nc.vector.tensor_scalar_add(
                    out=qhatT[:, c, nn:nn + 512], in0=ps, scalar1=bq_sb[:, c:c + 1]
                )

            def khat_chunk(c):
                for n0 in range(0, LKP, 512):
                    khat_slice(c, n0)

            def qhat_chunk(c):
                for nn in range(0, LQ, 512):
                    qhat_half(c, nn)

            def v_chunk(t):
                ps = psum_o.tile([128, DC], F32, name=f"ps_v{t}", tag="pso")
                for d in range(DK):
                    nc.tensor.matmul(
                        ps,
                        lhsT=kvT_in[:, d, t * 128:(t + 1) * 128],
                        rhs=wv_sb[:, d, :],
                        start=(d == 0), stop=(d == DK - 1),
                    )
                nc.vector.tensor_copy(
                    out=v_sb[:, t, :].rearrange("p (h e) -> p h e", e=E)[:, :, 0:DH],
                    in_=ps.rearrange("p (h e) -> p h e", e=DH),
                )

            def attention_pair(hp, qh, fillers=()):
                """One head pair (2 heads at partitions 0-63/64-127), one q half.
                j loop is software-pipelined: PV for step j is emitted after the
                scores for step j+1, so exp latency never stalls the PE."""
                fillers = list(fillers)
                h0, h1 = 2 * hp, 2 * hp + 1
                q0 = qh * 512
                opsA = psum_o.tile([E, 512], F32, name=f"opsA{hp}_{qh}", tag="pso")
                opsB = psum_o.tile([E, 512], F32, name=f"opsB{hp}_{qh}", tag="pso")
                NJP = (KT + 1) // 2
                es_t = [None] * NJP

                def pv_quad(jp):
                    # one es-wait covers up to 4 matmuls (pipelined entries)
                    for h, ops, o0 in ((h0, opsA, 0), (h1, opsB, 512)):
                        for i in range(2 if 2 * jp + 1 < KT else 1):
                            j = 2 * jp + i
                            nc.tensor.matmul(
                                ops,
                                lhsT=v_sb[:, j, h * E:(h + 1) * E],
                                rhs=es_t[jp][:, i, o0:o0 + 512],
                                start=(j == 0), stop=(j == KT - 1),
                            )

                def scores_exp(j):
                    ps = psum_big.tile([128, 1024], F32, name=f"ps_s{hp}_{qh}_{j}", tag="ss")
                    # head pair in disjoint PE row groups -> concurrent
                    nc.tensor.matmul(
                        ps[:, 0:512],
                        lhsT=khatT[0:64, hp, j * 128:(j + 1) * 128],
                        rhs=qhatT[0:64, hp, q0:q0 + 512],
                        start=True, stop=True,
                    )
                    nc.tensor.matmul(
                        ps[:, 512:1024],
                        lhsT=khatT[64:128, hp, j * 128:(j + 1) * 128],
                        rhs=qhatT[64:128, hp, q0:q0 + 512],
                        start=True, stop=True,
                    )
                    if fillers:
                        fillers.pop(0)()
                    jp, i = j // 2, j % 2
                    if i == 0:
                        es_t[jp] = exps_pool.tile(
                            [128, 2, 1024], F16, name=f"es{hp}_{qh}_{jp}", tag="es")
                    nc.scalar.activation(
                        out=es_t[jp][:, i, :], in_=ps,
                        func=mybir.ActivationFunctionType.Exp,
                        bias=mask_sb[:, j:j + 1], scale=1.0,
                    )

                for j in range(KT):
                    scores_exp(j)
                    if j % 2 == 1 and j >= 3:
                        pv_quad(j // 2 - 1)
                if KT % 2 == 1:
                    pv_quad((KT - 1) // 2 - 1)
                pv_quad((KT - 1) // 2)

                for h, po, ops in ((h0, 0, opsA), (h1, 64, opsB)):
                    # drain PSUM on ACT (frees the pso slot without waiting on
                    # the busy vector queue)
                    ou = small.tile([E, 512], F32, name=f"ou{h}_{qh}", tag="ou")
                    nc.scalar.copy(out=ou, in_=ops)
                    den = small.tile([1, 512], F32, name=f"den{h}_{qh}", tag="den")
                    nc.vector.tensor_copy(out=den, in_=ou[DH:DH + 1, :])
                    rec = small.tile([1, 512], F32, name=f"rec{h}_{qh}", tag="rec")
                    nc.vector.reciprocal_approx_fast(out=rec, in_=den)
                    rbc = small.tile([64, 512], F32, name=f"rbc{h}_{qh}", tag="rbc")
                    nc.gpsimd.partition_broadcast(rbc, rec)
                    nc.vector.tensor_mul(
                        out=onormT[po:po + 64, hp, q0:q0 + 512],
                        in0=ou[0:DH, :], in1=rbc,
                    )
                while fillers:
                    fillers.pop(0)()

            def wo_tile(m, drain):
                ys = yout.tile([128, 1024], F16, name=f"ys{m}", tag="ys")
                for n in range(2):
                    ps = psum_o.tile([128, 512], F32, name=f"ps_y{m}_{n}", tag="pso")
                    for c in range(DCH):
                        nc.tensor.matmul(
                            ps,
                            lhsT=onormT[:, c, m * 128:(m + 1) * 128],
                            rhs=wo_sb[:, c, n * 512:(n + 1) * 512],
                            start=(c == 0), stop=(c == DCH - 1),
                        )
                    drain(out=ys[:, n * 512:(n + 1) * 512], in_=ps)
                eng = nc.sync if m % 2 == 0 else nc.scalar
                eng.dma_start(out=y_d[m * 128:(m + 1) * 128, :], in_=ys)

            # ---- emission order ----
            khat_slice(0, 0)
            qhat_half(0, 0)

            def proj_fillers(c):
                f = [lambda n0=n0: khat_slice(c, n0) for n0 in range(0, LKP, 512)]
                f += [lambda nn=nn: qhat_half(c, nn) for nn in range(0, LQ, 512)]
                return f

            # qh=0 sweep: pair hp's fillers prefetch chunk hp+1's projections
            # (pair 0 also computes the V chunks, needed by its own PV steps).
            lead = [lambda n0=n0: khat_slice(0, n0) for n0 in range(512, LKP, 512)]
            lead += proj_fillers(1)
            lead += [lambda: qhat_half(0, 512)]

            def slot0(t, extras):
                def f():
                    v_chunk(t)
                    for e in extras:
                        e()
                return f

            nlead = len(lead)
            p0f = []
            for t in range(KT):
                take = lead[:2] if t < nlead / 2 + 1 else lead[:1]
                lead = lead[len(take):]
                p0f.append(slot0(t, take))
            p0f += lead
            attention_pair(0, 0, fillers=p0f)
            for hp in range(1, DCH):
                f = proj_fillers(hp + 1) if hp + 1 < DCH else []
                attention_pair(hp, 0, fillers=f)
            # qh=1 sweep: fill with W_O tiles for the first q half (PSUM drain
            # on the vector engine; ACT is busy with exp there).
            wof = [
                lambda m=m: wo_tile(m, nc.vector.tensor_copy)
                for m in range(QT // 2)
            ]
            attention_pair(0, 1, fillers=wof[0:1])
            attention_pair(1, 1, fillers=wof[1:2])
            attention_pair(2, 1, fillers=wof[2:3])
            attention_pair(3, 1, fillers=wof[3:4])
            # tail: W_O tiles for the second q half (drains split vector/ACT)
            for m in range(QT // 2, QT):
                drain = nc.vector.tensor_copy if m % 2 == 0 else (
                    lambda out, in_: nc.scalar.copy(out=out, in_=in_))
                wo_tile(m, drain)

    nc.compile()
    return nc


def _get_program(KT):
    key = ("nc", KT)
    if key not in _CACHE:
        _CACHE[key] = _build_program(KT)
    return _CACHE[key]


def kernel(q_input, kv_input, key_padding_mask, W_Q, b_Q, W_K, b_K, W_V, b_V, W_O, b_O):
    q_input = np.asarray(q_input, dtype=np.float32)
    kv_input = np.asarray(kv_input, dtype=np.float32)
    key_padding_mask = np.asarray(key_padding_mask).astype(bool)
    W_Q = np.asarray(W_Q, dtype=np.float32)
    b_Q = np.asarray(b_Q, dtype=np.float32)
    W_K = np.asarray(W_K, dtype=np.float32)
    W_V = np.asarray(W_V, dtype=np.float32)
    b_V = np.asarray(b_V, dtype=np.float32)
    W_O = np.asarray(W_O, dtype=np.float32)
    b_O = np.asarray(b_O, dtype=np.float32)

    # compact keys/values to the unmasked rows, pad to a 128 multiple
    keeps = [~key_padding_mask[b] for b in range(B)]
    effs = [int(k.sum()) for k in keeps]
    KT = max(2, math.ceil(max(effs) / 128))
    LKP = KT * 128
    nc = _get_program(KT)

    # per head-group constants
    hg_const = []
    for hg in range(2):
        sl = slice(hg * DC, (hg + 1) * DC)
        wq = np.ascontiguousarray(
            (W_Q[:, sl] * 0.125).astype(NP_F16).reshape(DK, 128, DCH, 128).transpose(2, 1, 0, 3))
        wk = np.ascontiguousarray(
            W_K[:, sl].astype(NP_F16).reshape(DK, 128, DCH, 128).transpose(2, 1, 0, 3))
        wv = np.ascontiguousarray(W_V[:, sl].astype(NP_F16).reshape(DK, 128, DC).transpose(1, 0, 2))
        wo = np.ascontiguousarray(W_O[sl, :].astype(NP_F16).reshape(DCH, 128, D).transpose(1, 0, 2))
        bq = np.ascontiguousarray((b_Q[sl] * 0.125).astype(np.float32).reshape(DCH, 128).T)
        hg_const.append((wq, wk, wv, wo, bq))

    per_batch = []
    for b in range(B):
        kvcT = kv_input[b][keeps[b]].T       # [D, eff] f32
        kvT = np.zeros((128, DK, LKP), NP_F16)
        kvT[:, :, :effs[b]] = kvcT.astype(NP_F16).reshape(DK, 128, effs[b]).transpose(1, 0, 2)
        maskb = np.full(KT * 128, np.float32(NEG), np.float32)
        maskb[:effs[b]] = np.float32(SHIFT)
        mask2 = np.ascontiguousarray(maskb.reshape(KT, 128).T)   # [128, KT]
        qTp = np.ascontiguousarray(
            q_input[b].T.astype(NP_F16).reshape(DK, 128, LQ).transpose(1, 0, 2))
        per_batch.append((qTp, kvT, mask2))

    in_maps = []
    for core in range(N_CORES):
        b, hg = core // 2, core % 2
        wq, wk, wv, wo, bq = hg_const[hg]
        qT, kvT, mask2 = per_batch[b]
        in_maps.append({
            "qT": qT, "kvT": kvT,
            "wq": wq, "wk": wk, "wv": wv, "wo": wo,
            "bq": bq, "maskb": mask2,
        })

    _CACHE["in_maps"] = in_maps
    _CACHE["last_KT"] = KT
    res = run_bass_kernel_spmd(nc, in_maps, core_ids=list(range(N_CORES)))
    bias_full = (b_V.astype(np.float64) @ W_O.astype(np.float64) + b_O).astype(np.float32)
    out = np.stack(
        [res.results[2 * b]["y"].astype(np.float32) + res.results[2 * b + 1]["y"].astype(np.float32)
         for b in range(B)]
    ) + bias_full
    return out.astype(np.float32)
